# revision 2
# baseline (speedup 1.0000x reference)
"""Trainium2 Bass kernel for per-gene linear layer.

Math (reference):
    gene    = x[:, :20000]           # (B, G)
    nongene = x[:, 20000:]           # (B, K=128)
    y[:, g] = gene[:, g] * W[g, 0] + nongene @ W[g, 1:] + b[g]

Sharding: model parallel over genes across 8 cores (2500 genes each).
Per-core device layout keeps genes on the partition axis ([G, B] output),
so the diagonal term is a per-partition fused multiply-add on DVE and the
bias is a K=1 rank-1 matmul accumulated into PSUM.

Per gene tile (128 genes x 1024 batch):
    psum  = b_row.T @ ones + wshT.T @ xnT        (TensorE, accumulated)
    out   = xg * dw[:, None] + psum              (VectorE, one fused op)
"""

import os
import numpy as np
from contextlib import ExitStack

import concourse.bass as bass
import concourse.tile as tile
from concourse import bacc, mybir
from concourse.bass_utils import run_bass_kernel_spmd

B = 1024           # batch
G = 20000          # genes (output dim)
K = 128            # shared nongene features
IN_DIM = G + K     # 20128
N_CORES = 8
G_CORE = G // N_CORES            # 2500 genes per core
N_GT = (G_CORE + 127) // 128     # 20 gene tiles per core
LAST_P = G_CORE - (N_GT - 1) * 128   # 68 partitions in last tile

_NC_CACHE = None
LAST_RESULTS = None  # BassKernelResults of the most recent run (for test harness)


def _build_nc():
    nc = bacc.Bacc("TRN2", target_bir_lowering=False, debug=False,
                   enable_asserts=True, num_devices=N_CORES)
    f32 = mybir.dt.float32

    xgT = nc.dram_tensor("xgT", [G_CORE, B], f32, kind="ExternalInput").ap()
    wshT = nc.dram_tensor("wshT", [K, G_CORE], f32, kind="ExternalInput").ap()
    xnT = nc.dram_tensor("xnT", [K, B], f32, kind="ExternalInput").ap()
    dwt = nc.dram_tensor("dwt", [128, N_GT], f32, kind="ExternalInput").ap()
    brow = nc.dram_tensor("brow", [1, G_CORE], f32, kind="ExternalInput").ap()
    yT = nc.dram_tensor("yT", [G_CORE, B], f32, kind="ExternalOutput").ap()

    with tile.TileContext(nc) as tc, ExitStack() as ctx:
        const = ctx.enter_context(tc.tile_pool(name="const", bufs=1))
        xg_pool = ctx.enter_context(tc.tile_pool(name="xg", bufs=3))
        out_pool = ctx.enter_context(tc.tile_pool(name="out", bufs=3))
        psum_pool = ctx.enter_context(
            tc.tile_pool(name="psum", bufs=3, space="PSUM"))

        wsh_s = const.tile([K, G_CORE], f32)
        nc.sync.dma_start(wsh_s[:], wshT[:])
        xn_s = const.tile([K, B], f32)
        nc.sync.dma_start(xn_s[:], xnT[:])
        dw_s = const.tile([128, N_GT], f32)
        nc.sync.dma_start(dw_s[:], dwt[:])
        b_s = const.tile([1, G_CORE], f32)
        nc.sync.dma_start(b_s[:], brow[:])
        ones_s = const.tile([1, 512], f32)
        nc.gpsimd.memset(ones_s[:], 1.0)

        for gt in range(N_GT):
            P = 128 if gt < N_GT - 1 else LAST_P
            g0 = gt * 128

            xg = xg_pool.tile([128, B], f32)
            nc.sync.dma_start(xg[:P, :], xgT[g0:g0 + P, :])

            psum = psum_pool.tile([128, B], f32)
            for h in range(2):
                c0 = h * 512
                # bias: rank-1 b_row.T @ ones -> adds b[g] to every column
                nc.tensor.matmul(psum[:P, c0:c0 + 512],
                                 b_s[0:1, g0:g0 + P], ones_s[:, :],
                                 start=True, stop=False)
                # shared-weight term: wshT.T @ xnT
                nc.tensor.matmul(psum[:P, c0:c0 + 512],
                                 wsh_s[:, g0:g0 + P], xn_s[:, c0:c0 + 512],
                                 start=False, stop=True)

            out = out_pool.tile([128, B], f32)
            # out = (xg * dw) + psum  -- one fused DVE pass
            nc.vector.scalar_tensor_tensor(
                out[:P, :], xg[:P, :], dw_s[:P, gt:gt + 1], psum[:P, :],
                op0=mybir.AluOpType.mult, op1=mybir.AluOpType.add)

            nc.sync.dma_start(yT[g0:g0 + P, :], out[:P, :])

    nc.compile()
    return nc


def _get_nc():
    global _NC_CACHE
    if _NC_CACHE is None:
        _NC_CACHE = _build_nc()
    return _NC_CACHE


def kernel(x, W, b):
    global LAST_RESULTS
    x = np.asarray(x, dtype=np.float32)
    W = np.asarray(W, dtype=np.float32)
    b = np.asarray(b, dtype=np.float32)
    assert x.shape == (B, IN_DIM) and W.shape == (G, 1 + K) and b.shape == (G,)

    xT = np.ascontiguousarray(x.T)          # (20128, 1024)
    xnT = np.ascontiguousarray(xT[G:])      # (128, 1024), replicated

    full = (N_GT - 1) * 128
    in_maps = []
    for c in range(N_CORES):
        g0 = c * G_CORE
        Wc = W[g0:g0 + G_CORE]
        dw = np.ascontiguousarray(Wc[:, 0])
        dwt = np.zeros((128, N_GT), np.float32)
        dwt[:, :N_GT - 1] = dw[:full].reshape(N_GT - 1, 128).T
        dwt[:LAST_P, N_GT - 1] = dw[full:]
        in_maps.append({
            "xgT": xT[g0:g0 + G_CORE],
            "wshT": np.ascontiguousarray(Wc[:, 1:].T),
            "xnT": xnT,
            "dwt": dwt,
            "brow": np.ascontiguousarray(b[g0:g0 + G_CORE]).reshape(1, G_CORE),
        })

    nc = _get_nc()
    trace = bool(os.environ.get("KERNEL_TRACE"))
    kwargs = {}
    if trace:
        tdir = os.environ.get("KERNEL_TRACE_DIR")
        if tdir:
            os.makedirs(tdir, exist_ok=True)
            kwargs["tmpdir"] = tdir
    LAST_RESULTS = run_bass_kernel_spmd(nc, in_maps, list(range(N_CORES)),
                                        trace=trace, **kwargs)
    yT = np.concatenate([LAST_RESULTS.results[c]["yT"] for c in range(N_CORES)],
                        axis=0)
    return np.ascontiguousarray(yT.T)


# revision 6
# speedup vs baseline: 2.1300x; 2.1300x over previous
"""Trainium2 Bass kernel for per-gene linear layer.

Math (reference):
    gene    = x[:, :20000]           # (B, G)
    nongene = x[:, 20000:]           # (B, K=128)
    y[:, g] = gene[:, g] * W[g, 0] + nongene @ W[g, 1:] + b[g]

Sharding: model parallel over genes across 8 cores (2500 genes each).
Per-core device layout keeps genes on the partition axis ([G, B] output),
so the diagonal term is a per-partition fused multiply-add on DVE and the
bias is a K=1 rank-1 matmul accumulated into PSUM.

Per gene tile (128 genes x 1024 batch):
    psum  = wshT.T @ xnT            (TensorE, float32r: full rate, ~tf32 precision)
    t     = psum + b[:, None]       (ScalarE activation copy, per-partition bias)
    out   = xg * dw[:, None] + t    (VectorE, one fused scalar_tensor_tensor)
"""

import os
import numpy as np
from contextlib import ExitStack

import concourse.bass as bass
import concourse.tile as tile
from concourse import bacc, mybir
from concourse.bass_utils import run_bass_kernel_spmd

B = 1024           # batch
G = 20000          # genes (output dim)
K = 128            # shared nongene features
IN_DIM = G + K     # 20128
N_CORES = 8
G_CORE = G // N_CORES            # 2500 genes per core
N_GT = (G_CORE + 127) // 128     # 20 gene tiles per core
LAST_P = G_CORE - (N_GT - 1) * 128   # 68 partitions in last tile

_NC_CACHE = None
LAST_RESULTS = None  # BassKernelResults of the most recent run (for test harness)


def _build_nc():
    nc = bacc.Bacc("TRN2", target_bir_lowering=False, debug=False,
                   enable_asserts=True, num_devices=N_CORES)
    f32 = mybir.dt.float32
    f32r = mybir.dt.float32r  # 4-byte storage, reduced-precision PE mode

    xgT = nc.dram_tensor("xgT", [G_CORE, B], f32, kind="ExternalInput").ap()
    wshT = nc.dram_tensor("wshT", [K, G_CORE], f32r, kind="ExternalInput").ap()
    xnT = nc.dram_tensor("xnT", [K, B], f32r, kind="ExternalInput").ap()
    dwt = nc.dram_tensor("dwt", [128, N_GT], f32, kind="ExternalInput").ap()
    bt = nc.dram_tensor("bt", [128, N_GT], f32, kind="ExternalInput").ap()
    yT = nc.dram_tensor("yT", [G_CORE, B], f32, kind="ExternalOutput").ap()

    with tile.TileContext(nc) as tc, ExitStack() as ctx:
        const = ctx.enter_context(tc.tile_pool(name="const", bufs=1))
        xg_pool = ctx.enter_context(tc.tile_pool(name="xg", bufs=3))
        t_pool = ctx.enter_context(tc.tile_pool(name="t", bufs=3))
        out_pool = ctx.enter_context(tc.tile_pool(name="out", bufs=3))
        psum_pool = ctx.enter_context(
            tc.tile_pool(name="psum", bufs=3, space="PSUM"))

        wsh_s = const.tile([K, G_CORE], f32r)
        nc.sync.dma_start(wsh_s[:], wshT[:])
        xn_s = const.tile([K, B], f32r)
        nc.sync.dma_start(xn_s[:], xnT[:])
        dw_s = const.tile([128, N_GT], f32)
        nc.sync.dma_start(dw_s[:], dwt[:])
        b_s = const.tile([128, N_GT], f32)
        nc.sync.dma_start(b_s[:], bt[:])

        for gt in range(N_GT):
            P = 128 if gt < N_GT - 1 else LAST_P
            g0 = gt * 128

            xg = xg_pool.tile([128, B], f32)
            nc.sync.dma_start(xg[:P, :], xgT[g0:g0 + P, :])

            psum = psum_pool.tile([128, B], f32)
            for h in range(2):
                c0 = h * 512
                # shared-weight term: wshT.T @ xnT (float32r -> full PE rate)
                nc.tensor.matmul(psum[:P, c0:c0 + 512],
                                 wsh_s[:, g0:g0 + P], xn_s[:, c0:c0 + 512],
                                 start=True, stop=True)

            # t = psum + b  (ScalarE PSUM->SBUF move with per-partition bias)
            t = t_pool.tile([128, B], f32)
            nc.scalar.activation(t[:P, :], psum[:P, :],
                                 mybir.ActivationFunctionType.Identity,
                                 bias=b_s[:P, gt:gt + 1], scale=1.0)

            out = out_pool.tile([128, B], f32)
            # out = (xg * dw) + t  -- one fused DVE pass, all-SBUF
            nc.vector.scalar_tensor_tensor(
                out[:P, :], xg[:P, :], dw_s[:P, gt:gt + 1], t[:P, :],
                op0=mybir.AluOpType.mult, op1=mybir.AluOpType.add)

            nc.sync.dma_start(yT[g0:g0 + P, :], out[:P, :])

    nc.compile()
    return nc


def _get_nc():
    global _NC_CACHE
    if _NC_CACHE is None:
        _NC_CACHE = _build_nc()
    return _NC_CACHE


def kernel(x, W, b):
    global LAST_RESULTS
    x = np.asarray(x, dtype=np.float32)
    W = np.asarray(W, dtype=np.float32)
    b = np.asarray(b, dtype=np.float32)
    assert x.shape == (B, IN_DIM) and W.shape == (G, 1 + K) and b.shape == (G,)

    xT = np.ascontiguousarray(x.T)          # (20128, 1024)
    xnT = np.ascontiguousarray(xT[G:])      # (128, 1024), replicated

    full = (N_GT - 1) * 128
    in_maps = []
    for c in range(N_CORES):
        g0 = c * G_CORE
        Wc = W[g0:g0 + G_CORE]

        def cols(v):
            m = np.zeros((128, N_GT), np.float32)
            m[:, :N_GT - 1] = v[:full].reshape(N_GT - 1, 128).T
            m[:LAST_P, N_GT - 1] = v[full:]
            return m

        in_maps.append({
            "xgT": xT[g0:g0 + G_CORE],
            "wshT": np.ascontiguousarray(Wc[:, 1:].T),
            "xnT": xnT,
            "dwt": cols(np.ascontiguousarray(Wc[:, 0])),
            "bt": cols(np.ascontiguousarray(b[g0:g0 + G_CORE])),
        })

    nc = _get_nc()
    trace = bool(os.environ.get("KERNEL_TRACE"))
    kwargs = {}
    if trace:
        tdir = os.environ.get("KERNEL_TRACE_DIR")
        if tdir:
            os.makedirs(tdir, exist_ok=True)
            kwargs["tmpdir"] = tdir
    LAST_RESULTS = run_bass_kernel_spmd(nc, in_maps, list(range(N_CORES)),
                                        trace=trace, **kwargs)
    yT = np.concatenate([LAST_RESULTS.results[c]["yT"] for c in range(N_CORES)],
                        axis=0)
    return np.ascontiguousarray(yT.T)
